# revision 2
# baseline (speedup 1.0000x reference)
"""LoRA linear (y = x @ (W + s*B@A)^T + bias) on 8 Trainium2 NeuronCores.

Strategy: data parallel over tokens (2048/core), LoRA folded into W on host
(W' = W + 4*B@A), GEMM run in fp8e4 (e4m3) with MatmulPerfMode.DoubleRow.

DoubleRow packs two 128-deep contraction slices per instruction ([128p,2,f]
operands, K=256/instr) and streams 4 fp8 bytes/lane/cycle: measured ~131ns
per 512-col instruction vs ~259ns for fp32r/bf16/fp16 - a 4x MAC rate. e4m3
alone is too coarse (fro err 3.8e-2 > 2e-2 gate), so each side is split
hi+lo: xs = xh + xl, ws = wh + wl with xh = e4m3(xs), xl = e4m3(xs - xh)
(shared power-of-2 scale per side - fp8 is floating point, so the residual
needs no separate scale). y*sx*sw = xh@wh + xh@wl + xl@wh (the xl@wl term is
~1e-6 relative, dropped): 3 GEMM-equivalents at 1/4 rate = 0.75x the fp32r
instruction budget, measured fro err 1.1e-3.

Orientation: w stationary ([128d,2,128o] slices), x moving ([128d,2,512n]),
psum [128o, 512n] - so bias is per-PARTITION and the whole eviction fuses
into ONE pass: scalar engine activation(Identity, scale=1/(sx*sw),
bias=bias[o]) or DVE tensor_scalar(mult, add), alternated per tile to halve
the tail. Output leaves as f16 [1024o, 2048n] per core; host transposes.

Schedule per core: 4 n-blocks x 8 o-tiles x 12 k-steps (4 kb x 3 terms,
term order hh, lh(wl*xh), hl(wh*xl) so x-lo streams last). First 3 n-blocks
run k-step-outer (each arriving tile enables 8 matmuls during the DMA fill);
the last runs o-tile-outer so psum completions (and evictions) spread out
instead of bursting at the end. All input DMAs ride one sequencer (sync) in
exact consumption order; outputs ride gpsimd/sync; bias rides scalar.
"""

import os
import sys

import numpy as np

for _p in ("/opt/trn_rl_repo", "/opt/pypackages"):
    if os.path.isdir(_p) and _p not in sys.path:
        sys.path.append(_p)

try:
    import jax

    jax.config.update(
        "jax_compilation_cache_dir", os.path.expanduser("~/.cache/jax_bass_cache")
    )
    jax.config.update("jax_persistent_cache_min_compile_time_secs", 0.0)
except Exception:
    pass

try:
    # bass_utils imports this when tracing is requested via BASS_TRACE; the
    # agent image ships a stub antenv without it. Register a no-op fallback
    # so a trace request degrades to "no trace" instead of crashing.
    from antenv import axon_hooks as _axon_hooks  # noqa: F401
except ImportError:
    import types as _types

    import antenv as _antenv

    _hooks = _types.ModuleType("antenv.axon_hooks")
    _hooks._hook = None
    _hooks.set_axon_ntff_profile_hook = lambda h: setattr(_hooks, "_hook", h)
    _hooks.get_axon_ntff_profile_hook = lambda: _hooks._hook
    sys.modules["antenv.axon_hooks"] = _hooks
    _antenv.axon_hooks = _hooks

import ml_dtypes  # noqa: E402

import concourse.bass as bass  # noqa: E402,F401
import concourse.mybir as mybir  # noqa: E402
import concourse.tile as tile  # noqa: E402
from concourse import bacc  # noqa: E402
from concourse.bass_utils import run_bass_kernel_spmd  # noqa: E402

N_CORES = 8
N_TOK, D_IN, D_OUT = 16384, 1024, 1024
NS = N_TOK // N_CORES  # 2048 tokens per core
P = 128
KB = D_IN // 256  # 4 k-blocks of 256 (DoubleRow pairs)
NB = NS // 512  # 4 n-blocks of 512 tokens
OT = D_OUT // P  # 8 o-tiles
NW = 512  # moving cols per instruction (psum bank)
SCALING = 4.0  # alpha / r = 32 / 8
SX = 16.0  # x pre-scale (power of 2; x ~ N(0,1), absmax*16 << 240)

F8 = ml_dtypes.float8_e4m3

_CACHE: dict = {}


def build_nc(inv_scale: float):
    f32 = mybir.dt.float32
    f16 = mybir.dt.float16
    bf16 = mybir.dt.bfloat16
    f8 = mybir.dt.float8e4
    DR = mybir.MatmulPerfMode.DoubleRow
    nc = bacc.Bacc("TRN2", target_bir_lowering=False, debug=False)

    xh_d = nc.dram_tensor("xh", [KB, P, NB, 2, NW], f8, kind="ExternalInput")
    xl_d = nc.dram_tensor("xl", [KB, P, NB, 2, NW], f8, kind="ExternalInput")
    wh_d = nc.dram_tensor("wh", [KB, P, 2, D_OUT], f8, kind="ExternalInput")
    wl_d = nc.dram_tensor("wl", [KB, P, 2, D_OUT], f8, kind="ExternalInput")
    bias_d = nc.dram_tensor("biasT", [P, OT], f32, kind="ExternalInput")
    out_d = nc.dram_tensor("outT", [D_OUT, NS], f16, kind="ExternalOutput")

    with tile.TileContext(nc) as tc:
        with tc.tile_pool(name="const", bufs=1) as cp, \
                tc.tile_pool(name="xp", bufs=4 * KB) as xp, \
                tc.tile_pool(name="op", bufs=8) as op, \
                tc.tile_pool(name="ps", bufs=8, space="PSUM") as pp:
            wh_t = [cp.tile([P, 2, D_OUT], f8, name=f"wh{k}") for k in range(KB)]
            wl_t = [cp.tile([P, 2, D_OUT], f8, name=f"wl{k}") for k in range(KB)]
            bias_sb = cp.tile([P, OT], f32)

            # Warm-up matmuls on zeroed bf16 scratch: occupy the PE during the
            # DMA fill so the clock-ramp (pstate) window is paid before real
            # operands land.
            warm_x = cp.tile([P, P], bf16)
            warm_w = cp.tile([P, NW], bf16)
            nc.gpsimd.memset(warm_x[:], 0.0)
            nc.gpsimd.memset(warm_w[:], 0.0)
            warm_ps = pp.tile([P, NW], f32, name="warm_ps", tag="psum")
            for _ in range(6):
                nc.tensor.matmul(warm_ps[:], warm_x[:], warm_w[:],
                                 start=True, stop=True)

            # bias rides the scalar sequencer, off the critical input queue
            nc.scalar.dma_start(bias_sb[:], bias_d[:])

            def x_tiles(nb):
                th = [xp.tile([P, 2, NW], f8, name=f"xh_n{nb}_k{k}", tag="xd")
                      for k in range(KB)]
                tl = [xp.tile([P, 2, NW], f8, name=f"xl_n{nb}_k{k}", tag="xd")
                      for k in range(KB)]
                return th, tl

            def load_x(tiles, dram, nb):
                th, tl = tiles
                for k in range(KB):
                    nc.sync.dma_start(th[k][:], dram[0][k][:, nb])
                for k in range(KB):
                    nc.sync.dma_start(tl[k][:], dram[1][k][:, nb])

            # Startup stream in exact consumption order: first (wh0 o-slice,
            # xh(0,0)) pair enables matmul #1 after ~160KB, then the rest of
            # the hh operands, then wl, xl(0), then later n-blocks.
            xt = {0: x_tiles(0)}
            th0, tl0 = xt[0]
            nc.sync.dma_start(wh_t[0][:, :, 0:2 * P], wh_d[0][:, :, 0:2 * P])
            nc.sync.dma_start(th0[0][:], xh_d[0][:, 0])
            nc.sync.dma_start(wh_t[0][:, :, 2 * P:D_OUT],
                              wh_d[0][:, :, 2 * P:D_OUT])
            for k in range(1, KB):
                nc.sync.dma_start(wh_t[k][:], wh_d[k][:])
                nc.sync.dma_start(th0[k][:], xh_d[k][:, 0])
            for k in range(KB):
                nc.sync.dma_start(wl_t[k][:], wl_d[k][:])
            for k in range(KB):
                nc.sync.dma_start(tl0[k][:], xl_d[k][:, 0])
            xt[1] = x_tiles(1)
            load_x(xt[1], (xh_d, xl_d), 1)

            # 12 k-steps: (stationary tiles, moving tiles) per term
            def schedule(nb):
                th, tl = xt[nb]
                steps = []
                for k in range(KB):
                    steps.append((wh_t[k], th[k]))
                for k in range(KB):
                    steps.append((wl_t[k], th[k]))
                for k in range(KB):
                    steps.append((wh_t[k], tl[k]))
                return steps

            def evict(nb, j, psum, out_q):
                o_sb = op.tile([P, NW], f16)
                if j % 2 == 0:
                    nc.scalar.activation(
                        o_sb[:], psum[:],
                        mybir.ActivationFunctionType.Identity,
                        bias=bias_sb[:, j:j + 1], scale=inv_scale,
                    )
                else:
                    nc.vector.tensor_scalar(
                        o_sb[:], psum[:], inv_scale, bias_sb[:, j:j + 1],
                        mybir.AluOpType.mult, mybir.AluOpType.add,
                    )
                out_q.dma_start(
                    out_d[j * P:(j + 1) * P, nb * NW:(nb + 1) * NW], o_sb[:]
                )

            NSTEP = 3 * KB
            for nb in range(NB):
                steps = schedule(nb)
                psums = [
                    pp.tile([P, NW], f32, name=f"ps_n{nb}_{j}", tag="psum")
                    for j in range(OT)
                ]
                if nb < NB - 1:
                    # k-step-outer: each arriving (w, x) pair immediately
                    # enables 8 matmuls while later slices are in flight.
                    for s, (wt, xtile) in enumerate(steps):
                        for j in range(OT):
                            nc.tensor.matmul(
                                psums[j][:], wt[:, :, j * P:(j + 1) * P],
                                xtile[:],
                                start=(s == 0), stop=(s == NSTEP - 1),
                                perf_mode=DR,
                            )
                    if nb + 2 < NB:
                        xt[nb + 2] = x_tiles(nb + 2)
                        load_x(xt[nb + 2], (xh_d, xl_d), nb + 2)
                    for j in range(OT):
                        evict(nb, j, psums[j], nc.gpsimd)
                else:
                    # last n-block, data resident: o-tile-outer spreads psum
                    # completions so the tail isn't 8 serialized evictions.
                    for j in range(OT):
                        for s, (wt, xtile) in enumerate(steps):
                            nc.tensor.matmul(
                                psums[j][:], wt[:, :, j * P:(j + 1) * P],
                                xtile[:],
                                start=(s == 0), stop=(s == NSTEP - 1),
                                perf_mode=DR,
                            )
                        evict(nb, j, psums[j],
                              nc.sync if j % 2 else nc.gpsimd)

    nc.finalize()
    return nc


def _get_nc(inv_scale: float):
    key = ("nc", inv_scale)
    if key not in _CACHE:
        _CACHE[key] = build_nc(inv_scale)
    return _CACHE[key]


def _split_f8(a: np.ndarray):
    """hi+lo e4m3 split of an f32 array (shared scale)."""
    hi = np.asarray(a, dtype=F8)
    lo = np.asarray(a - hi.astype(np.float32), dtype=F8)
    return hi, lo


def kernel(x, weight, bias, A, B):
    x = np.asarray(x, dtype=np.float32)
    weight = np.asarray(weight, dtype=np.float32)
    bias = np.asarray(bias, dtype=np.float32)
    A = np.asarray(A, dtype=np.float32)
    B = np.asarray(B, dtype=np.float32)

    # Fold the rank-8 LoRA update into the weight (exact up to fp32 rounding).
    w_eff = (
        weight.astype(np.float64)
        + SCALING * (B.astype(np.float64) @ A.astype(np.float64))
    ).astype(np.float32)

    # w pre-scale: power of 2 putting absmax in [60, 120] (e4m3 max 240).
    wmax = float(np.abs(w_eff).max()) or 1.0
    sw = 2.0 ** int(np.floor(np.log2(120.0 / wmax)))
    inv_scale = 1.0 / (SX * sw)

    # Device layouts. x: [KB, P, NB, 2, NW] per core with d = kb*256+i*128+p;
    # w: [KB, P, 2, D_OUT] with the same (kb, i, p) mapping.
    xs = np.ascontiguousarray(x.T) * np.float32(SX)  # [d, n]
    xh, xl = _split_f8(xs)
    xh = xh.reshape(KB, 2, P, N_TOK)
    xl = xl.reshape(KB, 2, P, N_TOK)

    ws = np.ascontiguousarray(w_eff.T) * np.float32(sw)  # [d, o]
    wh, wl = _split_f8(ws)
    wh = np.ascontiguousarray(
        wh.reshape(KB, 2, P, D_OUT).transpose(0, 2, 1, 3))
    wl = np.ascontiguousarray(
        wl.reshape(KB, 2, P, D_OUT).transpose(0, 2, 1, 3))
    biasT = np.ascontiguousarray(bias.reshape(OT, P).T)

    def core_x(a, c):
        v = a[:, :, :, c * NS:(c + 1) * NS].reshape(KB, 2, P, NB, NW)
        return np.ascontiguousarray(v.transpose(0, 2, 3, 1, 4))

    nc = _get_nc(inv_scale)
    in_maps = [
        {
            "xh": core_x(xh, c),
            "xl": core_x(xl, c),
            "wh": wh,
            "wl": wl,
            "biasT": biasT,
        }
        for c in range(N_CORES)
    ]
    trace_kwargs = {}
    if os.environ.get("KERNEL_TRACE") == "1":
        trace_kwargs = {"trace": True}
    res = run_bass_kernel_spmd(nc, in_maps, list(range(N_CORES)), **trace_kwargs)
    _CACHE["last_results"] = res
    return np.concatenate(
        [r["outT"].astype(np.float32).T for r in res.results], axis=0
    )


# revision 3
# speedup vs baseline: 1.4512x; 1.4512x over previous
"""LoRA linear (y = x @ (W + s*B@A)^T + bias) on 8 Trainium2 NeuronCores.

Strategy: data parallel over tokens (2048/core), LoRA folded into W on host
(W' = W + 4*B@A), single fp16 GEMM per core.

Why fp16: the PE streams ~1 moving column/cycle (~2.26 GHz) regardless of
operand dtype (fp32r == bf16 == fp16 == fp8 measured at ~227-236 ns per
512-col instruction), so a single GEMM is 131072 columns ~ 58 us steady no
matter what. fp8 DoubleRow halves instructions per GEMM but e4m3 alone fails
the 2e-2 gate (3.8e-2), and the needed hi+lo correction terms bring the
column count right back to 131072 - same speed, worse error, more traffic.
fp16 keeps full accuracy (fro err 2.9e-4) and halves HBM traffic vs fp32r
(x 4 MiB, w 2 MiB, out 4 MiB per core), which shrinks the startup fill.

Orientation: w stationary ([128d, 128o] slices of resident k-tiles), x
moving ([128d, 512n] fp16), psum [128o, 512n] - bias lands per-PARTITION so
the whole eviction is ONE fused pass: scalar-engine activation(Identity,
bias=bias[o]) or DVE tensor_scalar(add bias[o]), alternated per tile so the
tail drains on two engines. Output leaves as f16 [1024o, 2048n]; host
transposes back.

Schedule per core: 4 n-blocks x 8 o-tile psums x 8 k-steps. First 3
n-blocks run k-step-outer (each arriving (w,x) k-slice enables 8 matmuls
during the DMA fill); the last runs o-tile-outer so psum completions and
evictions spread across the final ~15 us instead of bursting. All input
DMAs ride one sequencer (sync) in exact consumption order (the fill is the
startup critical path); outputs ride gpsimd (+sync on the last block); bias
rides scalar. bf16 warm-up matmuls pre-pay the PE clock-ramp while the
first operands stream in, and a dummy Identity activation pre-loads the ACT
table off the critical path.
"""

import os
import sys

import numpy as np

for _p in ("/opt/trn_rl_repo", "/opt/pypackages"):
    if os.path.isdir(_p) and _p not in sys.path:
        sys.path.append(_p)

try:
    import jax

    jax.config.update(
        "jax_compilation_cache_dir", os.path.expanduser("~/.cache/jax_bass_cache")
    )
    jax.config.update("jax_persistent_cache_min_compile_time_secs", 0.0)
except Exception:
    pass

try:
    # bass_utils imports this when tracing is requested via BASS_TRACE; the
    # agent image ships a stub antenv without it. Register a no-op fallback
    # so a trace request degrades to "no trace" instead of crashing.
    from antenv import axon_hooks as _axon_hooks  # noqa: F401
except ImportError:
    import types as _types

    import antenv as _antenv

    _hooks = _types.ModuleType("antenv.axon_hooks")
    _hooks._hook = None
    _hooks.set_axon_ntff_profile_hook = lambda h: setattr(_hooks, "_hook", h)
    _hooks.get_axon_ntff_profile_hook = lambda: _hooks._hook
    sys.modules["antenv.axon_hooks"] = _hooks
    _antenv.axon_hooks = _hooks

import concourse.bass as bass  # noqa: E402,F401
import concourse.mybir as mybir  # noqa: E402
import concourse.tile as tile  # noqa: E402
from concourse import bacc  # noqa: E402
from concourse.bass_utils import run_bass_kernel_spmd  # noqa: E402

N_CORES = 8
N_TOK, D_IN, D_OUT = 16384, 1024, 1024
NS = N_TOK // N_CORES  # 2048 tokens per core
P = 128
KT = D_IN // P  # 8 k-tiles of 128
NB = NS // 512  # 4 n-blocks of 512 tokens
OT = D_OUT // P  # 8 o-tiles
NW = 512  # moving cols per instruction (one PSUM bank)
SCALING = 4.0  # alpha / r = 32 / 8

_CACHE: dict = {}


def build_nc():
    f32 = mybir.dt.float32
    f16 = mybir.dt.float16
    bf16 = mybir.dt.bfloat16
    nc = bacc.Bacc("TRN2", target_bir_lowering=False, debug=False)

    x_d = nc.dram_tensor("xT", [KT, P, NB, NW], f16, kind="ExternalInput")
    w_d = nc.dram_tensor("wT", [KT, P, D_OUT], f16, kind="ExternalInput")
    bias_d = nc.dram_tensor("biasT", [P, OT], f32, kind="ExternalInput")
    out_d = nc.dram_tensor("outT", [D_OUT, NS], f16, kind="ExternalOutput")

    with tile.TileContext(nc) as tc:
        with tc.tile_pool(name="const", bufs=1) as cp, \
                tc.tile_pool(name="xp", bufs=2 * KT) as xp, \
                tc.tile_pool(name="op", bufs=8) as op, \
                tc.tile_pool(name="ps", bufs=8, space="PSUM") as pp:
            w_t = [cp.tile([P, D_OUT], f16, name=f"w{k}") for k in range(KT)]
            bias_sb = cp.tile([P, OT], f32)

            # Warm-up matmuls on zeroed bf16 scratch: occupy the PE during
            # the DMA fill so the clock-ramp window is paid before real
            # operands land. The dummy activation pre-loads the ACT table.
            warm_x = cp.tile([P, P], bf16)
            warm_w = cp.tile([P, NW], bf16)
            warm_o = cp.tile([P, 1], f32)
            nc.gpsimd.memset(warm_x[:], 0.0)
            nc.gpsimd.memset(warm_w[:], 0.0)
            warm_ps = pp.tile([P, NW], f32, name="warm_ps", tag="psum")
            for _ in range(6):
                nc.tensor.matmul(warm_ps[:], warm_x[:], warm_w[:],
                                 start=True, stop=True)
            nc.scalar.activation(
                warm_o[:], warm_x[:, 0:1],
                mybir.ActivationFunctionType.Identity, bias=0.0, scale=1.0,
            )

            # bias rides the scalar sequencer, off the critical input queue
            nc.scalar.dma_start(bias_sb[:], bias_d[:])

            def x_tiles(nb):
                return [
                    xp.tile([P, NW], f16, name=f"x_n{nb}_k{k}", tag="xd")
                    for k in range(KT)
                ]

            def load_x(tiles, nb):
                for k in range(KT):
                    nc.sync.dma_start(tiles[k][:], x_d[k][:, nb])

            # Startup stream in exact consumption order: the first (w0
            # o-slice, x(0,0)) pair enables matmul #1 after ~160 KB.
            xt = {0: x_tiles(0)}
            nc.sync.dma_start(w_t[0][:, 0:2 * P], w_d[0][:, 0:2 * P])
            nc.sync.dma_start(xt[0][0][:], x_d[0][:, 0])
            nc.sync.dma_start(w_t[0][:, 2 * P:D_OUT], w_d[0][:, 2 * P:D_OUT])
            for k in range(1, KT):
                nc.sync.dma_start(w_t[k][:], w_d[k][:])
                nc.sync.dma_start(xt[0][k][:], x_d[k][:, 0])
            xt[1] = x_tiles(1)
            load_x(xt[1], 1)

            def evict(nb, j, psum, out_q):
                o_sb = op.tile([P, NW], f16)
                if j % 2 == 0:
                    nc.scalar.activation(
                        o_sb[:], psum[:],
                        mybir.ActivationFunctionType.Identity,
                        bias=bias_sb[:, j:j + 1], scale=1.0,
                    )
                else:
                    nc.vector.tensor_scalar(
                        o_sb[:], psum[:], bias_sb[:, j:j + 1], None,
                        mybir.AluOpType.add,
                    )
                out_q.dma_start(
                    out_d[j * P:(j + 1) * P, nb * NW:(nb + 1) * NW], o_sb[:]
                )

            for nb in range(NB):
                psums = [
                    pp.tile([P, NW], f32, name=f"ps_n{nb}_{j}", tag="psum")
                    for j in range(OT)
                ]
                if nb < NB - 1:
                    # k-step-outer: each arriving (w, x) k-slice immediately
                    # enables 8 matmuls while later slices are in flight.
                    for k in range(KT):
                        for j in range(OT):
                            nc.tensor.matmul(
                                psums[j][:], w_t[k][:, j * P:(j + 1) * P],
                                xt[nb][k][:],
                                start=(k == 0), stop=(k == KT - 1),
                            )
                    if nb + 2 < NB:
                        xt[nb + 2] = x_tiles(nb + 2)
                        load_x(xt[nb + 2], nb + 2)
                    for j in range(OT):
                        evict(nb, j, psums[j], nc.gpsimd)
                else:
                    # last n-block, data resident: o-tile-outer spreads psum
                    # completions so the tail isn't 8 serialized evictions.
                    for j in range(OT):
                        for k in range(KT):
                            nc.tensor.matmul(
                                psums[j][:], w_t[k][:, j * P:(j + 1) * P],
                                xt[nb][k][:],
                                start=(k == 0), stop=(k == KT - 1),
                            )
                        evict(nb, j, psums[j],
                              nc.sync if j % 2 else nc.gpsimd)

    nc.finalize()
    return nc


def _get_nc():
    if "nc" not in _CACHE:
        _CACHE["nc"] = build_nc()
    return _CACHE["nc"]


def kernel(x, weight, bias, A, B):
    x = np.asarray(x, dtype=np.float32)
    weight = np.asarray(weight, dtype=np.float32)
    bias = np.asarray(bias, dtype=np.float32)
    A = np.asarray(A, dtype=np.float32)
    B = np.asarray(B, dtype=np.float32)

    # Fold the rank-8 LoRA update into the weight (exact up to fp32 rounding).
    w_eff = (
        weight.astype(np.float64)
        + SCALING * (B.astype(np.float64) @ A.astype(np.float64))
    ).astype(np.float32)

    # Device layouts: x [KT, P, NB, NW] per core (d = k*128 + p, contiguous
    # 1 KB runs per partition per tile); w [KT, P, D_OUT] (2 KB runs).
    xT16 = np.ascontiguousarray(x.T).astype(np.float16)  # [d, n]
    wT16 = np.ascontiguousarray(w_eff.T).astype(np.float16)  # [d, o]
    w_host = np.ascontiguousarray(wT16.reshape(KT, P, D_OUT))
    biasT = np.ascontiguousarray(bias.reshape(OT, P).T.astype(np.float32))

    nc = _get_nc()
    in_maps = [
        {
            "xT": np.ascontiguousarray(
                xT16[:, c * NS:(c + 1) * NS].reshape(KT, P, NB, NW)),
            "wT": w_host,
            "biasT": biasT,
        }
        for c in range(N_CORES)
    ]
    trace_kwargs = {}
    if os.environ.get("KERNEL_TRACE") == "1":
        trace_kwargs = {"trace": True}
    res = run_bass_kernel_spmd(nc, in_maps, list(range(N_CORES)), **trace_kwargs)
    _CACHE["last_results"] = res
    return np.concatenate(
        [r["outT"].astype(np.float32).T for r in res.results], axis=0
    )
